# revision 46
# baseline (speedup 1.0000x reference)
"""Trainium2 Bass kernel for a linear-attention transformer block (fp8).

Model (see reference):
  ln1 -> q/k/v proj -> feature map elu(x)+1 -> linear attention via
  per-head kv summary [d,e] and k-sum [d] -> out proj -> residual ->
  ln2 -> MLP (gelu-tanh) -> residual.

Sharding (8 cores): token-parallel. Core c owns batch c//2, sequence half
c%2 (2048 tokens). Everything is token-local except the attention kv
summary (sum over the full sequence of a batch), which is reduced with a
pairwise AllReduce of a [128, 520] bf16 buffer (16 heads x [64, 65]
(kv | ksum), packed two heads per 128 partitions).

Device layout notes:
 - All six big matmul families (q/k/v/o projections, MLP fc/proj) run in
   fp8 e4m3 with perf_mode=DoubleRow: each matmul contracts TWO 128-row
   K-tiles at once (3D APs [128, 2, n]), halving PE column-stream time.
   Weights are absmax-scaled to power-of-two factors on the host; the
   dequant scale is folded into the activation-function `scale` argument
   or the fused (psum*s + residual) DVE op, so dequant costs no extra
   instructions.
 - Activations quantize to fp8 for free at an op that already exists
   (transpose copy-out, feature-map add, gelu). LN outputs (|x| <= ~5.2)
   and gelu outputs (<= 3.8) sit far below the 240 fp8e4 max, so no
   device-side clipping is required. The attention output (~1e-2 absmax)
   is scaled by 2^8 before fp8 (folded into the normalizer reciprocal).
 - The kv summary accumulates directly in two PSUM banks across the whole
   of pass A (memset once, every matmul start=False), with the ksum
   column folded in as a 65th all-ones column of the v tile.
 - Matmuls keep the 512-col moving operand wide so the 256-col DoubleRow
   LDWEIGHTS stays hidden: k/v/o stream weights against per-ts stationary
   activations; q/fc/proj stream a full 512-token block.
 - The MLP runs fc for a whole block into a 2MB fp8 h buffer, then proj
   in two output-half phases of 4 PSUM banks each.
 - q projections for all 4 blocks are emitted right after the collective
   trigger so the AllReduce latency (~35us) is fully hidden.
"""

import os
import sys
from contextlib import ExitStack

import numpy as np

for _p in ("/opt/trn_rl_repo",):
    if _p not in sys.path:
        sys.path.insert(0, _p)

import ml_dtypes  # noqa: E402

import concourse.bass as bass  # noqa: E402
import concourse.tile as tile  # noqa: E402
from concourse import bacc  # noqa: E402
from concourse import mybir  # noqa: E402
from concourse.masks import make_identity  # noqa: E402

BF16 = mybir.dt.bfloat16
FP32 = mybir.dt.float32
FP8 = mybir.dt.float8e4
AF = mybir.ActivationFunctionType
ALU = mybir.AluOpType
DR = mybir.MatmulPerfMode.DoubleRow

# Model dims (fixed by the problem).
B, S, H = 4, 4096, 1024
NH, HD = 16, 64
MLP = 4096

HC = H // 128    # 8 contraction chunks over hidden dim
FO = H // 128    # 8 feature chunks (q feature-major)
MO = MLP // 128  # 32 mlp chunks
BLK = 512        # tokens per block
TS = BLK // 128  # 128-token subtiles per block

LN_EPS = 1e-5
ATT_EPS = 1e-6
ATT_SC = 256.0      # fp8 scale for the attention output (absmax ~1e-2)
KV_SC = 1.0 / 32.0  # fp8 scale for the kv summary (kv<=1310, ksum<=4600)


def build_kernel(nc, t_core, n_cores, dq, apply_bias=False, sim_single=False):
    """Emit the per-core program.

    dq: dict of dequant multipliers (1/weight_scale, with the attention
    fp8 scale folded into 'ow').
    """
    nblk = t_core // BLK
    groups = [[2 * i, 2 * i + 1] for i in range(n_cores // 2)]

    x_d = nc.dram_tensor("x", [t_core, H], FP32, kind="ExternalInput")
    qw_d = nc.dram_tensor("qw", [128, HC * H], FP8, kind="ExternalInput")
    kw_d = nc.dram_tensor("kw", [128, HC * H], FP8, kind="ExternalInput")
    vw_d = nc.dram_tensor("vw", [128, HC * H], FP8, kind="ExternalInput")
    ow_d = nc.dram_tensor("ow", [128, HC * H], FP8, kind="ExternalInput")
    fcw_d = nc.dram_tensor("fcw", [128, MO * HC * 128], FP8, kind="ExternalInput")
    pjw_d = nc.dram_tensor("projw", [128, MO * H], FP8, kind="ExternalInput")
    bias_d = {}
    if apply_bias:
        for nm, n in (("qb", H), ("kb", H), ("vb", H), ("ob", H),
                      ("fcb", MLP), ("projb", H)):
            bias_d[nm] = nc.dram_tensor(nm, [1, n], BF16, kind="ExternalInput")
    out_d = nc.dram_tensor("out", [t_core, H], FP32, kind="ExternalOutput")

    with tile.TileContext(nc) as tc, ExitStack() as ctx:
        consts = ctx.enter_context(tc.tile_pool(name="consts", bufs=1))
        wpool = ctx.enter_context(tc.tile_pool(name="wpool", bufs=1))
        acts = ctx.enter_context(tc.tile_pool(name="acts", bufs=2))
        dram = ctx.enter_context(tc.tile_pool(name="dram", bufs=1, space="DRAM"))
        # PSUM: big 4x[128,512]f32 + mid 2x[128,256]bf16 + small 2 banks
        # (kv-summary accumulators in pass A, apply tiles in pass B).
        psum = ctx.enter_context(tc.tile_pool(name="psum", bufs=2, space="PSUM"))

        # ---- constants ----------------------------------------------------
        ident = consts.tile([128, 128], BF16)
        make_identity(nc, ident)
        eps_ln = consts.tile([128, 1], FP32)
        nc.vector.memset(eps_ln, LN_EPS)
        if apply_bias:
            ones_row = consts.tile([1, 128], BF16)
            nc.vector.memset(ones_row, 1.0)
            ones_t = consts.tile([1, BLK], BF16)
            nc.vector.memset(ones_t, 1.0)
            bias_sb = {}
            for nm, n in (("qb", H), ("kb", H), ("vb", H), ("ob", H),
                          ("fcb", MLP), ("projb", H)):
                b_t = consts.tile([1, n], BF16, name=f"{nm}_sb")
                nc.sync.dma_start(out=b_t, in_=bias_d[nm][:, :])
                bias_sb[nm] = b_t

        # resident weights (fp8, 1MB each): k/v first (needed earliest)
        kw = wpool.tile([128, HC, H], FP8)
        nc.sync.dma_start(out=kw, in_=kw_d[:, :])
        vw = wpool.tile([128, HC, H], FP8)
        nc.sync.dma_start(out=vw, in_=vw_d[:, :])
        ow = wpool.tile([128, HC, H], FP8)
        nc.sync.dma_start(out=ow, in_=ow_d[:, :])
        qw = wpool.tile([128, HC, FO * 128], FP8)
        nc.sync.dma_start(out=qw, in_=qw_d[:, :])

        # persistent activations
        lnxT = [wpool.tile([128, HC, BLK], FP8, name=f"lnxT{b}")
                for b in range(nblk)]
        qfT = [wpool.tile([128, FO, BLK], FP8, name=f"qfT{b}")
               for b in range(nblk)]
        hblk = wpool.tile([128, MO, BLK], FP8, name="hblk")
        # v tiles carry a 65th all-ones column per head (the ksum column).
        vts = [wpool.tile([128, 2, NH * 65], FP8, name=f"vt{i}")
               for i in range(2)]
        for v in vts:
            nc.vector.memset(v, 1.0)

        # kv-summary accumulator: all 16 heads flat in columns on
        # partitions 0:64 ([d, 16*(e|ksum)]), fp32 in SBUF.
        kvacc = consts.tile([64, NH * 65], FP32)
        nc.vector.memset(kvacc, 0.0)

        def layernorm_bf16(xt, dst):
            """dst = (xt - mean) * rsqrt(var + eps), cast to bf16.

            rsqrt = exp(-0.5*ln(var+eps)): ln and exp share one ACT
            table set with the feature-map exp, so the Sqrt set (a
            ~2.7us table reload per switch) is never touched.
            """
            stats = acts.tile([128, 2, 6], FP32, tag="ln_stats", bufs=2)
            nc.vector.bn_stats(out=stats[:, 0, :], in_=xt[:, 0:512])
            nc.vector.bn_stats(out=stats[:, 1, :], in_=xt[:, 512:1024])
            mv = acts.tile([128, 2], FP32, tag="ln_mv", bufs=2)
            nc.vector.bn_aggr(out=mv, in_=stats)
            rstd = acts.tile([128, 1], FP32, tag="ln_rstd", bufs=2)
            nc.scalar.activation(out=rstd, in_=mv[:, 1:2], func=AF.Sqrt,
                                 bias=eps_ln, scale=1.0)
            nc.vector.reciprocal(out=rstd, in_=rstd)
            nc.vector.tensor_scalar(out=dst, in0=xt, scalar1=mv[:, 0:1],
                                    scalar2=rstd, op0=ALU.subtract,
                                    op1=ALU.mult)

        def transpose_chunks(src_bf16, dstT, ts_idx):
            """PE-transpose [128,1024] token-major into fp8 chunks of dstT."""
            for hc in range(HC):
                pt = psum.tile([128, 256], BF16, tag="mid", bufs=2, name="pt")
                nc.tensor.transpose(pt[:, 0:128],
                                    src_bf16[:, hc * 128:(hc + 1) * 128],
                                    ident)
                dst = dstT[:, hc, ts_idx * 128:ts_idx * 128 + 128]
                if hc % 2 == 0:
                    nc.vector.tensor_copy(dst, pt[:, 0:128])
                else:
                    nc.scalar.copy(out=dst, in_=pt[:, 0:128])

        def feature_map(ps, dst, n, s):
            """dst = elu(s*ps)+1 = min(exp(s*ps),1) + relu(s*ps), fp8 out.

            The min runs on gpsimd (immediate-scalar ops are Pool-legal)
            so DVE only pays for the final add.
            """
            e = acts.tile([128, n], BF16, tag="fm_e", bufs=2, name="fm_e")
            nc.scalar.activation(out=e, in_=ps, func=AF.Exp, scale=s)
            r = acts.tile([128, n], BF16, tag="fm_r", bufs=2, name="fm_r")
            nc.scalar.activation(out=r, in_=ps, func=AF.Relu, scale=s)
            nc.vector.scalar_tensor_tensor(out=dst, in0=e, scalar=1.0, in1=r,
                                           op0=ALU.min, op1=ALU.add)

        def emit_ln1(blk):
            """DMA + LN1 for all 4 subtiles of a block (DVE/ACT only)."""
            tiles = []
            for ts in range(TS):
                xt = acts.tile([128, H], FP32, tag="xin", bufs=3)
                r0 = blk * BLK + ts * 128
                nc.gpsimd.dma_start(out=xt, in_=x_d[r0:r0 + 128, :])
                lnx = acts.tile([128, H], BF16, tag="lnx", bufs=5)
                layernorm_bf16(xt, lnx)
                tiles.append(lnx)
            return tiles

        def emit_lnT(blk, tiles):
            for ts in range(TS):
                transpose_chunks(tiles[ts], lnxT[blk], ts)

        def emit_kv(blk):
            """k/v projections + kv-summary for a block.

            All 64 projection matmuls are emitted before any summary
            matmul: the PE never has to sit behind a summary that waits
            on the feature-map chain, which keeps it dense enough that
            HAM stays at full clock.
            """
            kfs = []
            for tsp in range(TS // 2):
                kf = acts.tile([128, 2, H], FP8, tag="kf", bufs=2)
                kfs.append(kf)
                vt = vts[tsp]
                for s in range(2):
                    ts = tsp * 2 + s
                    for which in range(2):  # 0 = k, 1 = v
                        wsb = kw if which == 0 else vw
                        for half in range(2):
                            pp = psum.tile([128, 512], FP32, tag="big",
                                           bufs=4, name="pp_kv")
                            for hcp in range(HC // 2):
                                nc.tensor.matmul(
                                    pp,
                                    lhsT=lnxT[blk][:, 2 * hcp:2 * hcp + 2,
                                                   ts * 128:ts * 128 + 128],
                                    rhs=wsb[:, 2 * hcp:2 * hcp + 2,
                                            half * 512:half * 512 + 512],
                                    start=(hcp == 0),
                                    stop=(hcp == HC // 2 - 1
                                          and not apply_bias),
                                    perf_mode=DR)
                            if apply_bias:
                                nc.tensor.matmul(
                                    pp, lhsT=ones_row,
                                    rhs=bias_sb["kb" if which == 0 else "vb"]
                                        [0:1, half * 512:half * 512 + 512],
                                    start=False, stop=True)
                            if which == 0:
                                dst = kf[:, s, half * 512:half * 512 + 512]
                                feature_map(pp, dst, 512, dq["kw"])
                            else:
                                # heads of this half, 65-strided in vt
                                # (col 64 of each head stays the ksum ones)
                                dview = vt[:, s, :].rearrange(
                                    "p (h c) -> p h c", c=65)
                                dst = dview[:, half * 8:half * 8 + 8, 0:64]
                                src = pp[:, :].rearrange(
                                    "p (h c) -> p h c", c=64)
                                nc.scalar.mul(dst, src, dq["vw"])
            # kv + ksum summaries: 4 heads per [64, 260] psum group,
            # then one DVE accumulate into kvacc per group.
            for tsp in range(TS // 2):
                kf, vt = kfs[tsp], vts[tsp]
                for g in range(NH // 4):
                    pkv = psum.tile([64, 4 * 65], FP32, tag="small", bufs=2,
                                    name="pkv")
                    for j in range(4):
                        h = 4 * g + j
                        nc.tensor.matmul(
                            pkv[:, j * 65:j * 65 + 65],
                            lhsT=kf[:, :, h * 64:h * 64 + 64],
                            rhs=vt[:, :, h * 65:h * 65 + 65],
                            start=(j == 0), stop=(j == 3),
                            perf_mode=DR)
                    acc = kvacc[:, g * 260:g * 260 + 260]
                    nc.vector.tensor_add(out=acc, in0=acc, in1=pkv)

        # ================== PASS A ========================================
        ln_tiles = emit_ln1(0)
        emit_lnT(0, ln_tiles)
        for blk in range(nblk):
            emit_kv(blk)
            if blk + 1 < nblk:
                nxt = emit_ln1(blk + 1)
                emit_lnT(blk + 1, nxt)

        # ================== AllReduce of kv summary over the seq pair =====
        kvacc_bf = consts.tile([64, NH * 65], BF16)
        nc.vector.tensor_copy(kvacc_bf, kvacc)
        if sim_single:
            kvred = kvacc_bf
        else:
            cc_in = dram.tile([64, NH * 65], BF16)
            cc_out = dram.tile([64, NH * 65], BF16)
            nc.gpsimd.dma_start(out=cc_in, in_=kvacc_bf)
            nc.gpsimd.collective_compute(
                "AllReduce", ALU.add, replica_groups=groups,
                ins=[cc_in.opt()], outs=[cc_out.opt()])
            kvred = consts.tile([64, NH * 65], BF16)
            nc.gpsimd.dma_start(out=kvred, in_=cc_out)

        # q projection for one block (overlaps the collective). The first
        # block's first two groups take the idle mid banks so they don't
        # wait for DVE to drain pass A's feature maps from the big banks.
        def q_proj(blk):
            for fo in range(FO):
                tag = "mid" if blk == 0 and fo < 2 else "big"
                pp = psum.tile([128, 512], FP32, tag=tag, bufs=4 - 2 * (tag == "mid"),
                               name="pp_q")
                for hcp in range(HC // 2):
                    nc.tensor.matmul(
                        pp,
                        lhsT=qw[:, 2 * hcp:2 * hcp + 2,
                                fo * 128:fo * 128 + 128],
                        rhs=lnxT[blk][:, 2 * hcp:2 * hcp + 2, :],
                        start=(hcp == 0),
                        stop=(hcp == HC // 2 - 1 and not apply_bias),
                        perf_mode=DR)
                if apply_bias:
                    nc.tensor.matmul(pp,
                                     lhsT=bias_sb["qb"][0:1, fo * 128:
                                                        fo * 128 + 128],
                                     rhs=ones_t[0:1, 0:BLK],
                                     start=False, stop=True)
                feature_map(pp, qfT[blk][:, fo, :], BLK, dq["qw"])

        # block-diagonal fp8 [d, (e|ksum)] pairs, scaled by KV_SC; odd
        # heads shift from partitions 0:64 to 64:128 (DVE handles the
        # differing partition bases). Emitted after every q projection so
        # its wait on the collective never blocks the q feature maps.
        def build_kvaug():
            kvaug = consts.tile([128, 8 * 130], FP8)
            nc.vector.memset(kvaug, 0.0)
            for hp in range(8):
                nc.vector.tensor_scalar_mul(
                    out=kvaug[0:64, hp * 130:hp * 130 + 65],
                    in0=kvred[:, (2 * hp) * 65:(2 * hp) * 65 + 65],
                    scalar1=KV_SC)
                nc.vector.tensor_scalar_mul(
                    out=kvaug[64:128, hp * 130 + 65:hp * 130 + 130],
                    in0=kvred[:, (2 * hp + 1) * 65:(2 * hp + 1) * 65 + 65],
                    scalar1=KV_SC)
            return kvaug

        q_proj(0)
        q_proj(1)
        q_proj(2)
        q_proj(3)
        kvaug = build_kvaug()

        # ================== PASS B (software-pipelined) ===================
        # araw holds num*KV_SC and den*KV_SC; we want attn*ATT_SC in fp8:
        #   rc = ATT_SC/(den*KV_SC + eps*KV_SC)
        #      = 1/(dn/ATT_SC + eps*KV_SC/ATT_SC)
        def emit_apply(blk):
            """Apply matmuls + normalizers; returns normalized attn tiles.

            Normalize is interleaved per-ts (so ts0's attn is ready as
            early as possible for the transposes) and split between DVE
            and ACT (Copy with per-partition AP scale is table-free).
            """
            attns = []
            for ts in range(TS):
                araw = acts.tile([128, 8 * 130], BF16, tag="araw", bufs=4)
                for hp in range(8):
                    pa = psum.tile([128, 130], FP32,
                                   tag=("small" if hp % 2 == 0 else "mid"),
                                   bufs=2, name="pa")
                    nc.tensor.matmul(
                        pa,
                        lhsT=qfT[blk][:, hp, ts * 128:ts * 128 + 128],
                        rhs=kvaug[:, hp * 130:hp * 130 + 130],
                        start=True, stop=True)
                    dst = araw[:, hp * 130:hp * 130 + 130]
                    if hp % 2 == 0:
                        nc.vector.tensor_copy(dst, pa)
                    else:
                        nc.scalar.copy(out=dst, in_=pa)
                dn = araw.rearrange("p (g c) -> p g c", c=65)[:, :, 64:65]
                rc = acts.tile([128, 16], FP32, tag="rc", bufs=4)
                nc.vector.tensor_scalar(
                    out=rc, in0=dn.rearrange("p g c -> p (g c)"),
                    scalar1=1.0 / ATT_SC, scalar2=ATT_EPS * KV_SC / ATT_SC,
                    op0=ALU.mult, op1=ALU.add)
                nc.vector.reciprocal(out=rc, in_=rc)
                attn = acts.tile([128, H], BF16, tag="attn", bufs=5)
                for h in range(NH):
                    if h % 2 == 0:
                        nc.vector.tensor_scalar_mul(
                            out=attn[:, h * HD:(h + 1) * HD],
                            in0=araw[:, h * 65:h * 65 + 64],
                            scalar1=rc[:, h:h + 1])
                    else:
                        nc.scalar.activation(
                            out=attn[:, h * HD:(h + 1) * HD],
                            in_=araw[:, h * 65:h * 65 + 64],
                            func=AF.Copy, scale=rc[:, h:h + 1])
                attns.append(attn)
            return attns

        def emit_attnT(attns):
            attnT = acts.tile([128, HC, BLK], FP8, tag="attnT", bufs=2)
            for ts in range(TS):
                transpose_chunks(attns[ts], attnT, ts)
            return attnT

        def emit_oproj_ln2(blk, attnT):
            xrs = []
            ln2s = []

            def o_proj_ts(ts):
                xt = acts.tile([128, H], FP32, tag="xin2", bufs=2, name="xt2")
                r0 = blk * BLK + ts * 128
                nc.scalar.dma_start(out=xt, in_=x_d[r0:r0 + 128, :])
                xr = acts.tile([128, H], BF16, tag="xr", bufs=6, name="xr")
                for half in range(2):
                    pp = psum.tile([128, 512], FP32, tag="big", bufs=4,
                                   name="pp_o")
                    for hcp in range(HC // 2):
                        nc.tensor.matmul(
                            pp,
                            lhsT=attnT[:, 2 * hcp:2 * hcp + 2,
                                       ts * 128:ts * 128 + 128],
                            rhs=ow[:, 2 * hcp:2 * hcp + 2,
                                   half * 512:half * 512 + 512],
                            start=(hcp == 0),
                            stop=(hcp == HC // 2 - 1 and not apply_bias),
                            perf_mode=DR)
                    if apply_bias:
                        nc.tensor.matmul(
                            pp, lhsT=ones_row,
                            rhs=bias_sb["ob"][0:1, half * 512:
                                              half * 512 + 512],
                            start=False, stop=True)
                    nc.vector.scalar_tensor_tensor(
                        out=xr[:, half * 512:half * 512 + 512],
                        in0=pp, scalar=dq["ow"],
                        in1=xt[:, half * 512:half * 512 + 512],
                        op0=ALU.mult, op1=ALU.add)
                xrs.append(xr)
                ln2 = acts.tile([128, H], BF16, tag="ln2", bufs=2)
                layernorm_bf16(xr, ln2)
                ln2s.append(ln2)

            ln2T = acts.tile([128, HC, BLK], FP8, tag="ln2T", bufs=1)
            o_proj_ts(0)
            o_proj_ts(1)
            transpose_chunks(ln2s[0], ln2T, 0)
            o_proj_ts(2)
            transpose_chunks(ln2s[1], ln2T, 1)
            o_proj_ts(3)
            transpose_chunks(ln2s[2], ln2T, 2)
            transpose_chunks(ln2s[3], ln2T, 3)
            return xrs, ln2T

        def emit_fc(blk, ln2T):
            for mop in range(MO // 2):
                fcw_c = acts.tile([128, 2, HC, 128], FP8, tag="fcw_c",
                                  bufs=3)
                nc.sync.dma_start(out=fcw_c,
                                  in_=fcw_d[:, 2 * mop * (HC * 128):
                                            (2 * mop + 2) * (HC * 128)])
                for p in range(2):
                    mo = 2 * mop + p
                    pfc = psum.tile([128, 512], FP32, tag="big", bufs=4,
                                    name="pfc")
                    for hcp in range(HC // 2):
                        nc.tensor.matmul(
                            pfc,
                            lhsT=fcw_c[:, p, 2 * hcp:2 * hcp + 2, :],
                            rhs=ln2T[:, 2 * hcp:2 * hcp + 2, :],
                            start=(hcp == 0),
                            stop=(hcp == HC // 2 - 1 and not apply_bias),
                            perf_mode=DR)
                    if apply_bias:
                        nc.tensor.matmul(
                            pfc,
                            lhsT=bias_sb["fcb"][0:1, mo * 128:mo * 128 + 128],
                            rhs=ones_t[0:1, 0:BLK],
                            start=False, stop=True)
                    nc.scalar.activation(out=hblk[:, mo, :], in_=pfc,
                                         func=AF.Gelu_apprx_tanh,
                                         scale=dq["fcw"])

        def emit_proj(blk, xrs):
            for half in range(2):
                pps = [psum.tile([128, 512], FP32, tag="big", bufs=4,
                                 name=f"pproj_{blk}_{half}_{i}")
                       for i in range(TS)]
                for mop in range(MO // 2):
                    pjw_c = acts.tile([128, 2, H], FP8, tag="pjw_c", bufs=6)
                    nc.sync.dma_start(out=pjw_c,
                                      in_=pjw_d[:, 2 * mop * H:
                                                (2 * mop + 2) * H])
                    for tsl in range(TS):
                        nc.tensor.matmul(
                            pps[tsl],
                            lhsT=hblk[:, 2 * mop:2 * mop + 2,
                                      tsl * 128:tsl * 128 + 128],
                            rhs=pjw_c[:, :, half * 512:half * 512 + 512],
                            start=(mop == 0),
                            stop=(mop == MO // 2 - 1 and not apply_bias),
                            perf_mode=DR)
                if apply_bias:
                    for tsl in range(TS):
                        nc.tensor.matmul(
                            pps[tsl], lhsT=ones_row,
                            rhs=bias_sb["projb"][0:1, half * 512:
                                                 half * 512 + 512],
                            start=False, stop=True)
                for tsl in range(TS):
                    outt = acts.tile([128, 512], FP32, tag="outt", bufs=4)
                    xrh = xrs[tsl][:, half * 512:half * 512 + 512]
                    if tsl % 2 == 0:
                        nc.vector.scalar_tensor_tensor(
                            out=outt, in0=pps[tsl], scalar=dq["projw"],
                            in1=xrh, op0=ALU.mult, op1=ALU.add)
                    else:
                        # scalar engine drains (and releases) the psum bank
                        # so DVE and ACT split the four bank releases.
                        nc.scalar.mul(outt, pps[tsl], dq["projw"])
                        nc.vector.tensor_add(out=outt, in0=outt, in1=xrh)
                    r0 = blk * BLK + tsl * 128
                    # split the last block's final writes across two
                    # queues so the output drain tail halves
                    if blk == nblk - 1 and half == 1 and tsl % 2 == 1:
                        deng = nc.sync
                    else:
                        deng = nc.gpsimd
                    deng.dma_start(
                        out=out_d[r0:r0 + 128, half * 512:half * 512 + 512],
                        in_=outt)

        # Pipeline: block b+1's apply fills the LN2->fc dependency tail of
        # block b, and its attn transposes slot in after fc (so fc never
        # waits on them); o-proj of b+1 then starts right after proj of b.
        attnT_next = emit_attnT(emit_apply(0))
        for blk in range(nblk):
            xrs, ln2T = emit_oproj_ln2(blk, attnT_next)
            attns_next = emit_apply(blk + 1) if blk + 1 < nblk else None
            emit_fc(blk, ln2T)
            if attns_next is not None:
                attnT_next = emit_attnT(attns_next)
            emit_proj(blk, xrs)


# ======================= host side =======================================

def _pow2_scale(absmax):
    """Largest power-of-2 s with absmax*s <= 224 (fp8e4 max is 240)."""
    return 2.0 ** int(np.floor(np.log2(224.0 / max(absmax, 1e-30))))


def _prep_weights(inputs):
    """Fold LN affine params into adjacent weights; quantize to fp8 with
    per-tensor power-of-2 scales; pre-lay-out for SBUF.

    Returns (tensor_dict, dequant_dict, apply_bias).
    """
    f32 = lambda k: np.asarray(inputs[k], np.float32)
    bf = ml_dtypes.bfloat16
    f8 = ml_dtypes.float8_e4m3

    ln1_w, ln1_b = f32("ln1_w"), f32("ln1_b")
    ln2_w, ln2_b = f32("ln2_w"), f32("ln2_b")

    out = {}
    dq = {}
    biases = {}
    scales = {}

    def q8(name, w):
        s = _pow2_scale(np.abs(w).max())
        scales[name] = s
        dq[name] = 1.0 / s
        return np.clip(w * s, -240.0, 240.0).astype(f8)

    def qkv_like(name, w, b, q_layout):
        we = ln1_w[:, None] * w
        be = b + ln1_b @ w
        if q_layout:  # [p][hc][fo][m]
            wd = we.reshape(HC, 128, FO, 128).transpose(1, 0, 2, 3).reshape(128, HC * H)
        else:         # [p][hc][n]
            wd = we.reshape(HC, 128, H).transpose(1, 0, 2).reshape(128, HC * H)
        return q8(name, wd), be

    out["qw"], biases["qb"] = qkv_like("qw", f32("q_w"), f32("q_b"), True)
    out["kw"], biases["kb"] = qkv_like("kw", f32("k_w"), f32("k_b"), False)
    out["vw"], biases["vb"] = qkv_like("vw", f32("v_w"), f32("v_b"), False)

    o_w = f32("o_w")
    out["ow"] = q8("ow", o_w.reshape(HC, 128, H).transpose(1, 0, 2).reshape(128, HC * H))
    biases["ob"] = f32("o_b")

    fc_w, fc_b = f32("fc_w"), f32("fc_b")
    fce = ln2_w[:, None] * fc_w
    fcbe = fc_b + ln2_b @ fc_w
    # [p][mo][hc][m]
    out["fcw"] = q8("fcw", fce.reshape(HC, 128, MO, 128).transpose(1, 2, 0, 3).reshape(128, MO * H))
    biases["fcb"] = fcbe

    pj_w = f32("proj_w")
    out["projw"] = q8("projw", pj_w.reshape(MO, 128, H).transpose(1, 0, 2).reshape(128, MO * H))
    biases["projb"] = f32("proj_b")

    # attention fp8 scale folds into the o-proj dequant
    dq["ow"] = dq["ow"] / ATT_SC

    apply_bias = any(np.any(b != 0.0) for b in biases.values())
    if apply_bias:
        # biases enter the psum as rank-1 bf16 updates; pre-scale to match
        # the quantized-operand units of each psum.
        bs = {
            "qb": scales["qw"], "kb": scales["kw"], "vb": scales["vw"],
            "ob": scales["ow"] * ATT_SC, "fcb": scales["fcw"],
            "projb": scales["projw"],
        }
        for nm, b in biases.items():
            out[nm] = (b * bs[nm]).reshape(1, -1).astype(bf)
    return out, dq, apply_bias


def _run(inputs, trace=False):
    from concourse.bass_utils import run_bass_kernel_spmd

    n_cores = 8
    t_core = B * S // n_cores  # 2048

    x = np.ascontiguousarray(np.asarray(inputs["x"], np.float32))
    wd, dq, apply_bias = _prep_weights(inputs)

    nc = bacc.Bacc(None, num_devices=n_cores, target_bir_lowering=False)
    build_kernel(nc, t_core, n_cores, dq, apply_bias=apply_bias)
    nc.compile()

    half = S // 2
    in_maps = []
    for c in range(n_cores):
        b, sh = c // 2, c % 2
        m = {"x": np.ascontiguousarray(x[b, sh * half:(sh + 1) * half, :])}
        m.update(wd)
        in_maps.append(m)

    res = run_bass_kernel_spmd(nc, in_maps, core_ids=list(range(n_cores)),
                               trace=trace)

    out = np.empty((B, S, H), np.float32)
    for c in range(n_cores):
        b, sh = c // 2, c % 2
        out[b, sh * half:(sh + 1) * half, :] = res.results[c]["out"]
    return out, res


def kernel(**inputs):
    return _run(inputs)[0]


if __name__ == "__main__":
    os.environ.setdefault("BASS_NEVER_TRACE", "1")
    import reference

    inputs = {k: np.asarray(v) for k, v in reference.setup_inputs().items()}
    got = kernel(**inputs)
    exp = np.asarray(reference.reference(**inputs))
    err = np.abs(got - exp).max() / np.abs(exp).max()
    print("Relative error:", err)
